# revision 24
# baseline (speedup 1.0000x reference)
"""Trainium2 Bass kernel for GatedGraphXBias (gnn_message_passing), v3.

Reference math per iteration (T=2048 notes, E=12 edge types, H=64):
    act[e]  = edge[e].T @ h                      # [T, H]
    a       = sum_e (act[e] + ba[e]) @ W[e] + bw # [T, 3H] -> az|ar|ah
    a      += x @ Win                            # hoisted input projection
    z       = sigmoid(az + h @ Uz)
    r       = sigmoid(ar + h @ Ur)
    h~      = tanh(ah + (r*h) @ Uh)
    h       = (1-z)*h + z*h~

Sequence-parallel over T across 8 cores (TL=256 notes each).

v3: the per-iteration AllGather of h (NRT collective through DRAM + reload,
~20us/iter of serial latency) is replaced by direct SBUF->SBUF
remote_dma_broadcast pushes: each core sends its new h slice [128, 2H] bf16
to XOR-neighbor j (j=1..7) writing position j of the receiver's resident
gather buffer.  Descriptor generation (~1us/send on the Pool sequencer) is
hoisted into the mm1 window; a single trigger_dma fires all 7 sends the
moment hnew lands in SBUF.  Receivers gate their mm1 on a remote-bumped
semaphore (+2/send, 14/fill), double-buffered A/B by fill parity.  The
XOR-position permutation of source-note chunks is folded into the per-core
host-side layout of edge_in / h0sb_in, discovered at runtime by a tiny
probe kernel (sigma_m(j) = which core's slice lands at position j).  Falls
back to the v2 collective path if the probe fails.

v2 retained: fp16 matmul operands (f32 PSUM), mm1 column-group pairing via
tile_position (2 edge types concurrently in PE cols 0-63/64-127), mm2 over
the pair-stacked K=128, slot-major mm1 for it>=1 so PSUM copies + mm2
overlap the mm1 stream, r-first gate tail.
"""

import sys

sys.path.insert(0, "/opt/trn_rl_repo")

import numpy as np
import concourse.bass as bass
import concourse.mybir as mybir
import concourse.tile as tile
from concourse import library_config
from concourse.bass_utils import run_bass_kernel_spmd
from concourse.library_overlay import lower_extended_insts
from concourse.masks import make_identity
from concourse.vector_clock import ScopedClock

E, T, H, IN = 12, 2048, 64, 128
M = 8  # cores
TL = T // M  # 256 local notes per core
NCH = T // 128  # 16 contraction chunks of 128 source notes
NSLOT = 3  # mm1 PSUM slots, each covering 2 edge-type pairs
F32 = mybir.dt.float32
BF16 = mybir.dt.float16  # fp16: 11-bit mantissa (bf16's 8 was too coarse here)
U8 = mybir.dt.uint8
SIG = mybir.ActivationFunctionType.Sigmoid
TANH = mybir.ActivationFunctionType.Tanh
COPY = mybir.ActivationFunctionType.Copy
# per-chunk edge column order: slot k holds [e=4k (A), e=4k+2 (A), e=4k+1 (B), e=4k+3 (B)]
E_ORDER = [0, 2, 1, 3, 4, 6, 5, 7, 8, 10, 9, 11]


class SplitDrainTileContext(tile.TileContext):
    """TileContext that limits every instruction to a single sync wait.

    This walrus build rejects >1 sync wait command on an instruction
    (setupSyncWait: "Too many sync wait commands"), so extra waits are
    peeled onto standalone same-engine NoOps emitted just before the
    instruction — semantically identical (the engine stream waits
    sequentially at the same program point).

    Also supports `inject_wait(inst_name, sem, value)`: a sem-ge wait
    materialized as a NoOp at COMMIT time (post-scheduling), so waits that
    are satisfied by REMOTE cores (remote_dma sem bumps) never reach the
    single-core scheduling sim, which would otherwise deadlock on them."""

    def inject_wait(self, inst_name: str, sem, value: int):
        if not hasattr(self, "_inj_waits"):
            self._inj_waits = {}
        self._inj_waits.setdefault(inst_name, []).append((sem, value))

    def inject_inc_after(self, inst_name: str, sem, value: int):
        """Emit `sem += value` as an EventSemaphore right AFTER inst_name in
        its engine stream (commit-time; engine queues execute in order, so
        the inc fires once the instruction completes).  Used instead of
        then_inc: the race detector allows only one update per regular
        instruction."""
        if not hasattr(self, "_inj_incs"):
            self._inj_incs = {}
        self._inj_incs.setdefault(inst_name, []).append((sem, value))

    def _commit_instruction(self, inst, lazy_reg_writes: bool = True):
        for sem, value in getattr(self, "_inj_waits", {}).pop(inst.name, ()):
            w = mybir.SyncWait(
                sync_type="semaphore",
                id=sem.num,
                ant_name=sem.name,
                wait_mode="sem-ge-imm",
                wait_value=value,
                wait_reg=None,
            )
            nop = mybir.InstNoOp(
                name=f"injwait-{self.nc.next_id()}",
                sync_info=mybir.SyncInfo(on_wait=[w], on_update=[]),
                bass_nofuse=True,
                engine=inst.engine,
            )
            super()._commit_instruction(nop, lazy_reg_writes=False)
        si = getattr(inst, "sync_info", None)
        if si is not None and len(si.on_wait) > 1:
            waits = list(si.on_wait)
            inst.sync_info = mybir.SyncInfo(
                on_wait=[waits[-1]], on_update=list(si.on_update)
            )
            for w in waits[:-1]:
                nop = mybir.InstNoOp(
                    name=f"splitwait-{self.nc.next_id()}",
                    sync_info=mybir.SyncInfo(on_wait=[w], on_update=[]),
                    bass_nofuse=True,
                    engine=inst.engine,
                )
                super()._commit_instruction(nop, lazy_reg_writes=False)
        super()._commit_instruction(inst, lazy_reg_writes)
        # running post-completion totals of every sem, in committed order
        if not hasattr(self, "_sem_totals"):
            self._sem_totals = {}
        si2 = getattr(inst, "sync_info", None)
        if si2 is not None:
            for u in si2.on_update:
                if u.update_mode == "sem-add-imm":
                    inc = u.update_value
                elif u.update_mode == "sem-inc":
                    inc = u.update_value if u.update_value else 1
                else:
                    continue
                self._sem_totals[u.id] = self._sem_totals.get(u.id, 0) + inc
        for sem, value in getattr(self, "_inj_incs", {}).pop(inst.name, ()):
            # The inc must fire only after `inst`'s DATA completes, not when
            # the sequencer reaches it — wait on inst's own completion sem
            # at its running total.
            waits = []
            if si2 is not None and si2.on_update:
                u0 = si2.on_update[0]
                waits = [
                    mybir.SyncWait(
                        sync_type="semaphore",
                        id=u0.id,
                        ant_name=u0.ant_name,
                        wait_mode="sem-ge-imm",
                        wait_value=self._sem_totals[u0.id],
                        wait_reg=None,
                    )
                ]
            u = mybir.SyncUpdate(
                sync_type="semaphore",
                id=sem.num,
                ant_name=sem.name,
                update_mode="sem-add-imm",
                update_value=value,
                update_reg=None,
            )
            ev = mybir.InstEventSemaphore(
                name=f"injinc-{self.nc.next_id()}",
                sync_info=mybir.SyncInfo(on_wait=waits, on_update=[u]),
                bass_nofuse=True,
                engine=inst.engine,
            )
            super()._commit_instruction(ev, lazy_reg_writes=False)
            self._sem_totals[sem.num] = self._sem_totals.get(sem.num, 0) + value

    def _drain_and_barrier(self, tick_clock, wait_clock):
        drain_inst = self.nc.sync.drain()
        wait_clock.add_sem_waits(
            drain_inst.ins, ScopedClock({None: tick_clock.global_clock})
        )
        si = drain_inst.ins.sync_info
        waits = list(si.on_wait) if si is not None else []
        upds = list(si.on_update) if si is not None else []
        if len(waits) > 1:
            drain_inst.ins.sync_info = mybir.SyncInfo(on_wait=waits[:1], on_update=upds)
            for w in waits[1:]:
                nop = self.nc.sync.nop(nofuse=True, hint="split_drain_waits")
                nop.ins.sync_info = mybir.SyncInfo(on_wait=[w], on_update=[])

        self.nc.all_engine_barrier()
        assert self.sems is not None
        popped = self.nc._tile_sem_poison_stack.pop()
        assert popped is self._sem_poison
        self.nc.clear_and_free_semaphores(list(self.sems.allocated().values()))
        self.nc.all_engine_barrier()


# ---------------------------------------------------------------------------
# runtime probe: discover sigma_m(j) = jax core whose broadcast slot j lands
# at position j of receiver m (XOR topology over the physical tpb ids)
# ---------------------------------------------------------------------------


def build_probe() -> bass.Bass:
    nc = bass.Bass(
        "TRN2",
        target_bir_lowering=False,
        debug=False,
        num_devices=M,
        dynamic_dma_scratch_size=16384,
    )
    val_in = nc.declare_dram_parameter("val_in", [128, 8], F32, isOutput=False)
    gath_out = nc.declare_dram_parameter("gath_out", [128, 8 * M], F32, isOutput=True)

    with SplitDrainTileContext(nc) as tc:
        with (
            tc.tile_pool(name="work", bufs=1) as wpool,
            tc.tile_pool(name="dram", bufs=1, space="DRAM") as dpool,
        ):
            rsem = nc.alloc_semaphore("probe_rsem")
            lsem = nc.alloc_semaphore("probe_lsem")
            isem = nc.alloc_semaphore("probe_isem")

            val_sb = wpool.tile([128, 8], F32, name="val")
            nc.sync.dma_start(out=val_sb[:], in_=val_in[:])
            gbuf = wpool.tile([128, 8 * M], F32, name="gbuf")
            nc.vector.tensor_copy(gbuf[:, 0:8], val_sb[:])
            nc.vector.sem_inc(isem, 1)

            bar_in = dpool.tile([1, 1], U8, name="bar_in")
            bar_out = dpool.tile([M, 1], U8, name="bar_out", addr_space="Shared")
            nc.gpsimd.wait_ge(isem, 1)
            nc.gpsimd.collective_compute(
                "AllGather",
                mybir.AluOpType.bypass,
                replica_groups=[list(range(M))],
                ins=[bar_in[:]],
                outs=[bar_out[:]],
            )
            bar_sb = wpool.tile([1, 1], U8, name="bar_sb")
            junk_sb = wpool.tile([1, 1], U8, name="junk_sb")
            nc.gpsimd.dma_start(out=bar_sb[:], in_=bar_out[0:1, :])
            nc.gpsimd.tensor_copy(junk_sb[:], bar_sb[:])

            nc.gpsimd.load_library(library_config.remote_dma)
            for j in range(1, M):
                rdests: list = [None] * M
                rdests[j] = (0, j)
                nc.gpsimd.remote_dma_broadcast(
                    out_ap=gbuf[:, j * 8 : (j + 1) * 8],
                    in_ap=val_sb[:],
                    remote_sem=rsem,
                    local_sem=lsem,
                    rdests=rdests,
                )
            nc.gpsimd.trigger_dma(count=None)

            st = nc.sync.dma_start(out=gath_out[:], in_=gbuf[:])
            tc.inject_wait(st.ins.name, rsem, 14)

    lower_extended_insts(nc)
    return nc


_SIGMA: "np.ndarray | None | bool" = None  # None=unknown, False=probe failed


def _get_sigma():
    global _SIGMA
    if _SIGMA is None:
        try:
            nc = build_probe()
            in_maps = [
                {"val_in": np.full((128, 8), float(m), np.float32)} for m in range(M)
            ]
            res = run_bass_kernel_spmd(nc, in_maps, list(range(M)))
            sig = np.zeros((M, M), int)
            for m in range(M):
                out = res.results[m]["gath_out"]
                for j in range(M):
                    blk = out[:, j * 8 : (j + 1) * 8]
                    v = blk[0, 0]
                    if not np.all(blk == v):
                        raise ValueError(f"non-uniform probe block m={m} j={j}")
                    sig[m, j] = int(v)
            for m in range(M):
                if sorted(sig[m]) != list(range(M)) or sig[m, 0] != m:
                    raise ValueError(f"bad probe row {m}: {sig[m]}")
                for j in range(M):
                    if sig[sig[m, j], j] != m:
                        raise ValueError(f"asymmetric probe at m={m} j={j}")
            _SIGMA = sig
        except Exception as e:  # noqa: BLE001
            print(f"kernel: rdma probe failed ({e!r}); falling back to collective")
            _SIGMA = False
    return _SIGMA


# ---------------------------------------------------------------------------
# main kernel
# ---------------------------------------------------------------------------


def build(
    iteration: int,
    reps: int = 1,
    ablate: frozenset = frozenset(),
    use_rdma: bool = True,
    debug_bufs: bool = False,
) -> bass.Bass:
    nc = bass.Bass(
        "TRN2",
        target_bir_lowering=False,
        debug=False,
        num_devices=M,
        dynamic_dma_scratch_size=32768 if use_rdma else 2048,
    )

    # Per-core inputs (host pre-arranged; in rdma mode edge_in/h0sb_in source
    # chunks are permuted per-core into XOR-position order by sigma):
    #   edge_in : [T, E*TL] fp16, row = source note, col = chunk-local
    #             [slotk: eA0 | eA1 | eB0 | eB1] blocks of TL (see E_ORDER)
    #   h0sb_in : [128, NCH*H] fp16  initial h in chunk-major SBUF layout
    #   hid_in  : [T, H] fp16        initial h, note-major (coll-ablate path)
    #   hT0_in  : [H, TL] f32        local initial hidden, transposed
    #   hT0b_in : [H, TL] fp16
    #   xT_in   : [IN, TL] fp16      local input features, transposed
    #   w_in    : [128, 6*3*H] fp16  pair-stacked W: col p*192+g*64+j is
    #             gate g of pair p; rows 0-63 = W[2p], 64-127 = W[2p+1]
    #   win_in  : [IN, 3H] fp16
    #   uzr_in  : [H, 2H] fp16
    #   uh_in   : [H, H] fp16
    #   bz/br/bh: [H, 1] f32         folded biases (bw + sum_e ba[e] @ W[e])
    edge_in = nc.declare_dram_parameter("edge_in", [T, E * TL], BF16, isOutput=False)
    h0sb_in = nc.declare_dram_parameter("h0sb_in", [128, NCH * H], BF16, isOutput=False)
    hid_in = nc.declare_dram_parameter("hid_in", [T, H], BF16, isOutput=False)
    hT0_in = nc.declare_dram_parameter("hT0_in", [H, TL], F32, isOutput=False)
    hT0b_in = nc.declare_dram_parameter("hT0b_in", [H, TL], BF16, isOutput=False)
    xT_in = nc.declare_dram_parameter("xT_in", [IN, TL], BF16, isOutput=False)
    w_in = nc.declare_dram_parameter("w_in", [128, 6 * 3 * H], BF16, isOutput=False)
    win_in = nc.declare_dram_parameter("win_in", [IN, 3 * H], BF16, isOutput=False)
    uzr_in = nc.declare_dram_parameter("uzr_in", [H, 2 * H], BF16, isOutput=False)
    uh_in = nc.declare_dram_parameter("uh_in", [H, H], BF16, isOutput=False)
    bz_in = nc.declare_dram_parameter("bz_in", [H, 1], F32, isOutput=False)
    br_in = nc.declare_dram_parameter("br_in", [H, 1], F32, isOutput=False)
    bh_in = nc.declare_dram_parameter("bh_in", [H, 1], F32, isOutput=False)
    h_out = nc.declare_dram_parameter("h_out", [TL, H], F32, isOutput=True)
    if debug_bufs:
        dbg_outs = {
            n: nc.declare_dram_parameter(n, [128, NCH * H], BF16, isOutput=True)
            for n in ("hbuf0_out", "hbuf1_out")
        }
        dbg_outs.update(
            {
                n: nc.declare_dram_parameter(n, [128, 2 * H], BF16, isOutput=True)
                for n in ("hnewb0_out", "hnewb1_out")
            }
        )

    # (sem, value, [remote matmul inst names]) per fill — validated post-lower
    fill_checks: list = []
    pool_checks: dict = {"lib": None, "bar_copy": None, "first_trig": None,
                         "preps": [], "trigs": []}

    with SplitDrainTileContext(nc) as tc:
        with (
            # comm pool FIRST: remote-DMA-addressed tiles must sit at low
            # SBUF offsets (the D2D cross-die desc path mishandles high
            # destination offsets; intra-die is fine)
            tc.tile_pool(name="comm", bufs=1) as commpool,
            tc.tile_pool(name="edge", bufs=1) as edge_pool,
            tc.tile_pool(name="const", bufs=1) as cpool,
            tc.tile_pool(name="work", bufs=1) as wpool,
            tc.tile_pool(name="psum", bufs=1, space="PSUM") as ppool,
            tc.tile_pool(name="dram", bufs=2, space="DRAM") as dpool,
        ):
            if use_rdma:
                # persistent gather buffers (remote-written, parity by fill)
                hbuf = [
                    commpool.tile([128, NCH * H], BF16, name=f"hbuf{p}")
                    for p in range(2)
                ]
                # persistent broadcast sources (parity by fill)
                hnewb = [
                    commpool.tile([128, 2 * H], BF16, name=f"hnewb{p}")
                    for p in range(2)
                ]
            # ---- constants / weights (loaded once) ----
            w_sb = cpool.tile([128, 6 * 3 * H], BF16)
            nc.sync.dma_start(out=w_sb[:], in_=w_in[:])
            uzr_sb = cpool.tile([H, 2 * H], BF16)
            nc.sync.dma_start(out=uzr_sb[:], in_=uzr_in[:])
            uh_sb = cpool.tile([H, H], BF16)
            nc.sync.dma_start(out=uh_sb[:], in_=uh_in[:])
            win_sb = cpool.tile([IN, 3 * H], BF16)
            nc.sync.dma_start(out=win_sb[:], in_=win_in[:])
            xT_sb = cpool.tile([IN, TL], BF16)
            nc.sync.dma_start(out=xT_sb[:], in_=xT_in[:])
            bz_sb = cpool.tile([H, 1], F32)
            nc.sync.dma_start(out=bz_sb[:], in_=bz_in[:])
            br_sb = cpool.tile([H, 1], F32)
            nc.sync.dma_start(out=br_sb[:], in_=br_in[:])
            bh_sb = cpool.tile([H, 1], F32)
            nc.sync.dma_start(out=bh_sb[:], in_=bh_in[:])
            id64 = cpool.tile([H, H], F32)
            make_identity(nc, id64[:])
            id64b = cpool.tile([H, H], BF16)
            make_identity(nc, id64b[:])

            if use_rdma:
                rsem = [nc.alloc_semaphore("rsemA"), nc.alloc_semaphore("rsemB")]
                lsem = nc.alloc_semaphore("lsem")
                hsem = nc.alloc_semaphore("hsem")
                # persistent gather buffers (remote-written, parity by fill)
                hbuf = [
                    cpool.tile([128, NCH * H], BF16, name=f"hbuf{p}") for p in range(2)
                ]
                # persistent broadcast sources (parity by fill)
                hnewb = [
                    cpool.tile([128, 2 * H], BF16, name=f"hnewb{p}") for p in range(2)
                ]
                # entry barrier: peers must be in-kernel before remote writes
                bar_in = dpool.tile([1, 1], U8, name="bar_in", tag="bar_in", bufs=1)
                bar_out = dpool.tile(
                    [M, 1], U8, name="bar_out", tag="bar_out", bufs=1,
                    addr_space="Shared",
                )
                nc.gpsimd.collective_compute(
                    "AllGather",
                    mybir.AluOpType.bypass,
                    replica_groups=[list(range(M))],
                    ins=[bar_in[:]],
                    outs=[bar_out[:]],
                )
                bar_sb = cpool.tile([1, 1], U8, name="bar_sb")
                junk_sb = cpool.tile([1, 1], U8, name="junk_sb")
                nc.gpsimd.dma_start(out=bar_sb[:], in_=bar_out[0:1, :])
                bcp = nc.gpsimd.tensor_copy(junk_sb[:], bar_sb[:])
                barsem = nc.alloc_semaphore("barsem")
                tc.inject_inc_after(bcp.ins.name, barsem, 1)
                pool_checks["bar_copy"] = bcp.ins.name
                # Tile-visible edge barrier-chain -> preps: the preps read
                # hnewb, so a bar_sb-sourced write into each hnewb forces the
                # scheduler to place the whole chain before the first fill's
                # desc-gen (and thus before the first trigger, which would
                # otherwise stall the Pool queue on barsem ahead of the cc).
                for p in range(2):
                    nc.gpsimd.tensor_copy(hnewb[p][0:1, 0:1], bar_sb[:])
                lib = nc.gpsimd.load_library(library_config.remote_dma)
                pool_checks["lib"] = lib.ins.name
                fills = [0, 0]  # fills into parity A/B so far
                fill_no = 0  # global fill counter (across reps)
                hsem_target = 0

            for rep in range(reps):
                # ---- h state first (so iteration 0 isn't queued behind the
                # edge load on the SP HWDGE ring) ----
                if use_rdma:
                    h0_sb = cpool.tile(
                        [128, NCH * H], BF16, name="h0sb"
                    ) if rep == 0 else h0_sb
                    nc.sync.dma_start(out=h0_sb[:], in_=h0sb_in[:])
                else:
                    h_sb = wpool.tile([128, NCH * H], BF16, name="h", tag="h", bufs=1)
                    nc.sync.dma_start(out=h_sb[:], in_=h0sb_in[:])
                hT_sb = wpool.tile([H, TL], F32, name="hT", tag="hT", bufs=2)
                nc.sync.dma_start(out=hT_sb[:], in_=hT0_in[:])
                hTb_sb = wpool.tile([H, TL], BF16, name="hTb", tag="hTb", bufs=2)
                nc.sync.dma_start(out=hTb_sb[:], in_=hT0b_in[:])

                # ---- resident edge shard: 16 chunk DMAs, alternating the
                # SP / ACT HWDGE rings so chunk c lands ~c*1.2us in and the
                # chunk-outer mm1 of iteration 0 streams right behind ----
                edge_sb = []
                for c in range(NCH):
                    et = edge_pool.tile(
                        [128, E * TL], BF16, name=f"edge_c{c}", tag=f"edge_c{c}"
                    )
                    edge_sb.append(et)
                for c in range(NCH):
                    eng = nc.sync if c % 2 == 0 else nc.scalar
                    eng.dma_start(
                        out=edge_sb[c][:],
                        in_=edge_in[c * 128 : (c + 1) * 128, :],
                    )

                for it in range(iteration):
                    last = it == iteration - 1
                    if use_rdma:
                        hsrc = h0_sb if it == 0 else hbuf[(fill_no - 1) % 2]
                    else:
                        hsrc = h_sb

                    # Gate pre-activation groups [H, TL]: folded input
                    # projection starts each, U-gate matmuls fold in, then
                    # the 18 pair-stacked mm2 matmuls accumulate.
                    az_ps = ppool.tile([H, TL], F32, tag="az", bufs=1)
                    ar_ps = ppool.tile([H, TL], F32, tag="ar", bufs=1)
                    ah_ps = ppool.tile([H, TL], F32, tag="ah", bufs=1)
                    for g, ps in enumerate((az_ps, ar_ps, ah_ps)):
                        nc.tensor.matmul(
                            ps[:],
                            lhsT=win_sb[:, g * H : (g + 1) * H],
                            rhs=xT_sb[:],
                            start=True,
                            stop=False,
                            skip_group_check=True,
                        )
                    for g, ps in enumerate((az_ps, ar_ps)):
                        nc.tensor.matmul(
                            ps[:],
                            lhsT=uzr_sb[:, g * H : (g + 1) * H],
                            rhs=hTb_sb[:],
                            start=False,
                            stop="mm2" in ablate or "mm1" in ablate,
                            skip_group_check=True,
                        )

                    # -- mm1: col-group tiled over e-pairs --
                    act_ps = []
                    for k in range(NSLOT):
                        act_k = ppool.tile(
                            [128, 2 * TL], F32, name=f"act{k}", tag=f"act{k}", bufs=1
                        )
                        act_ps.append(act_k)
                    if "mm1" not in ablate:
                        # iteration 0: chunk-outer (pipelines behind the edge
                        # load); later iterations: slot-outer, so slot 0's
                        # PSUM copy + mm2 overlap the remaining mm1 stream
                        if it == 0:
                            order = [(c, k) for c in range(NCH) for k in range(NSLOT)]
                        else:
                            order = [(c, k) for k in range(NSLOT) for c in range(NCH)]
                        remote_mms: list = []
                        wait_spec = None
                        if use_rdma and it > 0 and "comm" not in ablate:
                            p_read = (fill_no - 1) % 2
                            wait_spec = (rsem[p_read], 14 * fills[p_read])
                        for c, k in order:
                            lhs = hsrc[:, c * H : (c + 1) * H]
                            base = k * 4 * TL
                            mm_a = nc.tensor.matmul(
                                act_ps[k][0:H, :],
                                lhsT=lhs,
                                rhs=edge_sb[c][:, base : base + 2 * TL],
                                start=(c == 0),
                                stop=(c == NCH - 1),
                                tile_position=(0, 0),
                                skip_group_check=True,
                            )
                            mm_b = nc.tensor.matmul(
                                act_ps[k][H:128, :],
                                lhsT=lhs,
                                rhs=edge_sb[c][:, base + 2 * TL : base + 4 * TL],
                                start=(c == 0),
                                stop=(c == NCH - 1),
                                tile_position=(0, H),
                                skip_group_check=True,
                            )
                            if wait_spec is not None and c >= 2:
                                if not remote_mms:
                                    tc.inject_wait(
                                        mm_a.ins.name, wait_spec[0], wait_spec[1]
                                    )
                                remote_mms.append(mm_a.ins.name)
                                remote_mms.append(mm_b.ins.name)
                        if wait_spec is not None:
                            fill_checks.append(
                                (wait_spec[0], wait_spec[1], remote_mms)
                            )

                        # -- rdma: desc-gen for this fill's 7 sends, hidden
                        # under the mm1 window (Pool is otherwise idle) --
                        if use_rdma and not last and "comm" not in ablate:
                            p_fill = fill_no % 2
                            for j in range(1, M):
                                rdests: list = [None] * M
                                rdests[j] = (0, j)
                                pr = nc.gpsimd.remote_dma_broadcast(
                                    out_ap=hbuf[p_fill][:, 2 * j * H : (2 * j + 2) * H],
                                    in_ap=hnewb[p_fill][:],
                                    remote_sem=rsem[p_fill],
                                    local_sem=lsem,
                                    rdests=rdests,
                                )
                                if j == 1 and fill_no > 0:
                                    # pace desc-gen: my receipt of fill G-1
                                    # (rsem) proves every peer's mm1 at
                                    # iteration G-1 ran, i.e. peers consumed
                                    # my fill G-2 sends -> those SWDGE ring
                                    # entries are reclaimable.  Fill G-1's
                                    # entries are protected by ring capacity
                                    # (>= 2 fills at 32KB scratch).
                                    pp = (fill_no - 1) % 2
                                    tc.inject_wait(
                                        pr.ins.name, rsem[pp], 14 * fills[pp]
                                    )
                                pool_checks["preps"].append(pr.ins.name)

                        # -- act PSUM -> SBUF (bf16) + mm2 --
                        for k in range(NSLOT):
                            act_sb = wpool.tile(
                                [128, 2 * TL], BF16, tag=f"actsb{k}", bufs=2
                            )
                            if k == 1:
                                nc.scalar.activation(
                                    act_sb[:],
                                    act_ps[k][:],
                                    COPY,
                                )
                            else:
                                nc.vector.tensor_copy(act_sb[:], act_ps[k][:])
                            if "mm2" in ablate:
                                continue
                            for j in range(2):
                                p = 2 * k + j
                                rhs = act_sb[:, j * TL : (j + 1) * TL]
                                for g, ps in enumerate((az_ps, ar_ps, ah_ps)):
                                    nc.tensor.matmul(
                                        ps[:],
                                        lhsT=w_sb[
                                            :,
                                            p * 3 * H + g * H : p * 3 * H + (g + 1) * H,
                                        ],
                                        rhs=rhs,
                                        start=False,
                                        stop=(p == 5 and g != 2),
                                        skip_group_check=True,
                                    )
                    # -- gates --  (r first: it heads the rh -> Uh -> tanh
                    # critical chain; z's products are precomputed in its
                    # shadow so only 2 DVE ops remain after the tanh)
                    r_sb = wpool.tile([H, TL], F32, tag="r")
                    nc.scalar.activation(r_sb[:], ar_ps[:], SIG, bias=br_sb[:])
                    z_sb = wpool.tile([H, TL], F32, tag="z")
                    nc.scalar.activation(z_sb[:], az_ps[:], SIG, bias=bz_sb[:])
                    rh_sb = wpool.tile([H, TL], BF16, tag="rh")
                    nc.vector.tensor_mul(rh_sb[:], r_sb[:], hT_sb[:])
                    nc.tensor.matmul(
                        ah_ps[:],
                        lhsT=uh_sb[:],
                        rhs=rh_sb[:],
                        start=False,
                        stop=True,
                        skip_group_check=True,
                    )
                    omz_sb = wpool.tile([H, TL], F32, tag="omz")
                    nc.vector.tensor_scalar(
                        omz_sb[:], z_sb[:], -1.0, 1.0,
                        mybir.AluOpType.mult, mybir.AluOpType.add,
                    )  # 1-z
                    omzh_sb = wpool.tile([H, TL], F32, tag="omzh")
                    nc.vector.tensor_mul(omzh_sb[:], omz_sb[:], hT_sb[:])  # (1-z)*h
                    ht_sb = wpool.tile([H, TL], F32, tag="ht")
                    nc.scalar.activation(ht_sb[:], ah_ps[:], TANH, bias=bh_sb[:])

                    zd_sb = wpool.tile([H, TL], F32, tag="zd")
                    nc.vector.tensor_mul(zd_sb[:], z_sb[:], ht_sb[:])  # z*h~
                    hnewT_sb = wpool.tile([H, TL], F32, tag="hT", bufs=2)
                    nc.vector.tensor_add(hnewT_sb[:], zd_sb[:], omzh_sb[:])

                    if last:
                        # transpose f32 -> [TL, H] and store the output
                        hnew_sb = wpool.tile([128, 2 * H], F32, tag="hnew")
                        for half in range(2):
                            tr_ps = ppool.tile([128, H], F32, tag="tr")
                            nc.tensor.transpose(
                                tr_ps[:],
                                hnewT_sb[:, half * 128 : (half + 1) * 128],
                                id64[:],
                            )
                            nc.vector.tensor_copy(
                                hnew_sb[:, half * H : (half + 1) * H], tr_ps[:]
                            )
                        nc.sync.dma_start(
                            out=h_out[:].rearrange("(c p) j -> p c j", p=128),
                            in_=hnew_sb[:].rearrange("p (c j) -> p c j", c=2),
                        )
                    elif use_rdma:
                        # fp16 hT copy for the next iteration's U-gate matmul
                        hnTb_sb = wpool.tile([H, TL], BF16, tag="hTb", bufs=2)
                        nc.vector.tensor_copy(hnTb_sb[:], hnewT_sb[:])
                        p_fill = fill_no % 2
                        for half in range(2):
                            tr_ps = ppool.tile([128, H], F32, tag="tr")
                            nc.tensor.transpose(
                                tr_ps[:],
                                hnewT_sb[:, half * 128 : (half + 1) * 128],
                                id64[:],
                            )
                            if "comm" not in ablate:
                                cpv = nc.vector.tensor_copy(
                                    hnewb[p_fill][:, half * H : (half + 1) * H],
                                    tr_ps[:],
                                )
                                tc.inject_inc_after(cpv.ins.name, hsem, 1)
                            # self slice -> position 0 of the gather buffer
                            nc.scalar.activation(
                                hbuf[p_fill][:, half * H : (half + 1) * H],
                                tr_ps[:],
                                COPY,
                            )
                        if "comm" not in ablate:
                            hsem_target += 2
                            trig = nc.gpsimd.trigger_dma(count=None)
                            tc.inject_wait(trig.ins.name, hsem, hsem_target)
                            pool_checks["trigs"].append(trig.ins.name)
                            if pool_checks["first_trig"] is None:
                                pool_checks["first_trig"] = trig.ins.name
                                # peers must be in-kernel before the first
                                # remote write actually fires
                                tc.inject_wait(trig.ins.name, barsem, 1)
                            fills[p_fill] += 1
                            fill_no += 1
                        hT_sb = hnewT_sb
                        hTb_sb = hnTb_sb
                    else:
                        # v2 collective path
                        hnTb_sb = wpool.tile([H, TL], BF16, tag="hTb", bufs=2)
                        nc.vector.tensor_copy(hnTb_sb[:], hnewT_sb[:])
                        hnew_sb = wpool.tile([128, 2 * H], BF16, tag="hnewb")
                        for half in range(2):
                            tr_ps = ppool.tile([128, H], F32, tag="tr")
                            nc.tensor.transpose(
                                tr_ps[:],
                                hnewT_sb[:, half * 128 : (half + 1) * 128],
                                id64[:],
                            )
                            nc.vector.tensor_copy(
                                hnew_sb[:, half * H : (half + 1) * H], tr_ps[:]
                            )

                        if "coll" not in ablate:
                            ag_in = dpool.tile([TL, H], BF16, tag="ag_in")
                            for half in range(2):
                                eng = nc.sync if half == 0 else nc.scalar
                                eng.dma_start(
                                    out=ag_in[half * 128 : (half + 1) * 128, :],
                                    in_=hnew_sb[:, half * H : (half + 1) * H],
                                )
                            ag_out = dpool.tile(
                                [T, H], BF16, tag="ag_out", addr_space="Shared"
                            )
                            nc.gpsimd.collective_compute(
                                "AllGather",
                                mybir.AluOpType.bypass,
                                replica_groups=[list(range(M))],
                                ins=[ag_in[:]],
                                outs=[ag_out[:]],
                            )
                            gather_src = ag_out
                            warm_ps = ppool.tile(
                                [H, 2 * TL], F32, name="warm", tag="warm", bufs=1
                            )
                            for _ in range(24):
                                nc.tensor.matmul(
                                    warm_ps[:],
                                    lhsT=hnew_sb[:, 0:H],
                                    rhs=edge_sb[0][:, 0 : 2 * TL],
                                    start=True,
                                    stop=True,
                                    skip_group_check=True,
                                )
                        else:
                            gather_src = hid_in
                        if "hreload" not in ablate:
                            h_sb = wpool.tile([128, NCH * H], BF16, tag="h", bufs=1)
                            qch = NCH // 4
                            for qq in range(4):
                                eng = nc.sync if qq % 2 == 0 else nc.scalar
                                eng.dma_start(
                                    out=h_sb[
                                        :, qq * qch * H : (qq + 1) * qch * H
                                    ].rearrange("p (c j) -> p c j", c=qch),
                                    in_=gather_src[
                                        qq * qch * 128 : (qq + 1) * qch * 128, :
                                    ].rearrange("(c p) j -> p c j", p=128),
                                )
                        hT_sb = hnewT_sb
                        hTb_sb = hnTb_sb

            if use_rdma and debug_bufs:
                nc.sync.dma_start(out=dbg_outs["hbuf0_out"][:], in_=hbuf[0][:])
                nc.sync.dma_start(out=dbg_outs["hbuf1_out"][:], in_=hbuf[1][:])
                nc.sync.dma_start(out=dbg_outs["hnewb0_out"][:], in_=hnewb[0][:])
                nc.sync.dma_start(out=dbg_outs["hnewb1_out"][:], in_=hnewb[1][:])

    if use_rdma:
        _validate_stream_order(nc, fill_checks, pool_checks)
        lower_extended_insts(nc)
    return nc


def _validate_stream_order(nc, fill_checks, pool_checks):
    """The TileScheduler is free to reorder same-engine instructions; verify
    the final PE stream places each fill's injected rsem wait before every
    matmul that reads remote gather chunks, and the Pool stream places the
    library load / barrier chain before the first prep / trigger."""
    streams: dict = {}
    for fn in nc.m.functions:
        for bb in fn.blocks:
            for ins in bb.instructions:
                eng = getattr(ins, "engine", None)
                if eng is None:
                    continue
                streams.setdefault(eng, []).append(ins)
    pe = streams.get(mybir.EngineType.PE, [])
    pe_pos = {ins.name: i for i, ins in enumerate(pe)}
    for sem, value, mm_names in fill_checks:
        wpos = None
        for i, ins in enumerate(pe):
            si = getattr(ins, "sync_info", None)
            if si is None:
                continue
            for w in si.on_wait:
                if w.id == sem.num and w.wait_value == value:
                    wpos = i
                    break
            if wpos is not None:
                break
        assert wpos is not None, f"missing injected wait {sem.name}>={value}"
        for n in mm_names:
            assert pe_pos[n] > wpos, (
                f"matmul {n} (pos {pe_pos[n]}) precedes its rsem wait "
                f"{sem.name}>={value} (pos {wpos})"
            )
    pool = streams.get(mybir.EngineType.Pool, [])
    pool_pos = {ins.name: i for i, ins in enumerate(pool)}
    lib = pool_pos[pool_checks["lib"]]
    for pr in pool_checks["preps"]:
        assert pool_pos[pr] > lib, f"prep {pr} precedes library load"
    if pool_checks["first_trig"] is not None:
        # the barsem injected wait must sit directly before the first trigger
        ft = pool_pos[pool_checks["first_trig"]]
        has_barwait = any(
            w.ant_name == "barsem" and w.wait_value == 1
            for ins in pool[:ft]
            if getattr(ins, "sync_info", None) is not None
            for w in ins.sync_info.on_wait
        )
        assert has_barwait, "first trigger lacks preceding barsem wait"
    # triggers must follow their preps (7 per fill, FIFO)
    for i, tr in enumerate(pool_checks["trigs"]):
        for pr in pool_checks["preps"][7 * i : 7 * (i + 1)]:
            assert pool_pos[pr] < pool_pos[tr], f"prep {pr} after trigger {tr}"


def _host_prep(
    input, hidden, edge_matrix, ba, wz_wr_wh, uz_ur, uh, input_wzrh, bw, sigma="auto"
):
    """Pre-arrange full inputs into the per-core DMA layouts."""
    if isinstance(sigma, str) and sigma == "auto":
        sigma = _SIGMA if isinstance(_SIGMA, np.ndarray) else None
    bf = mybir.dt.np(BF16)
    x = np.asarray(input, np.float32)[0]  # [T, IN]
    h0 = np.ascontiguousarray(np.asarray(hidden, np.float32)[0])  # [T, H]
    edge = np.asarray(edge_matrix, np.float32)  # [E, T, T]
    ba = np.asarray(ba, np.float32)
    W = np.asarray(wz_wr_wh, np.float32)  # [E, H, 3H]
    uzr = np.ascontiguousarray(np.asarray(uz_ur, np.float32))
    uh_ = np.ascontiguousarray(np.asarray(uh, np.float32))
    win = np.ascontiguousarray(np.asarray(input_wzrh, np.float32))
    bw = np.asarray(bw, np.float32)

    # folded bias: bw + sum_e ba[e] @ W[e]
    btot = bw + np.einsum("eh,ehk->k", ba, W)  # [3H]
    bz = np.ascontiguousarray(btot[:H].reshape(H, 1))
    br = np.ascontiguousarray(btot[H : 2 * H].reshape(H, 1))
    bh = np.ascontiguousarray(btot[2 * H :].reshape(H, 1))

    # edge shards: per core m, [T, E*TL] with columns in E_ORDER blocks
    # esh[m][s, i*TL + tl] = edge[E_ORDER[i], s, m*TL + tl]
    eperm = edge[E_ORDER]  # [E, T, T]
    esh = np.ascontiguousarray(
        eperm.reshape(E, T, M, TL).transpose(2, 1, 0, 3)
    ).reshape(M, T, E * TL)

    # pair-stacked W: w_flat[row, p*192 + g*64 + j]
    #   rows 0-63 = W[2p][:, g*64+j], rows 64-127 = W[2p+1][:, g*64+j]
    w_pair = np.empty((128, 6, 3 * H), np.float32)
    for p in range(6):
        w_pair[:H, p] = W[2 * p]
        w_pair[H:, p] = W[2 * p + 1]
    w_flat = np.ascontiguousarray(w_pair.reshape(128, 6 * 3 * H)).astype(bf)

    h0_chunks = h0.reshape(NCH, 128, H)  # global chunk-major

    in_maps = []
    for m in range(M):
        xT = np.ascontiguousarray(x[m * TL : (m + 1) * TL, :].T)
        hT0 = np.ascontiguousarray(h0[m * TL : (m + 1) * TL, :].T)
        if sigma is not None:
            perm = [int(p) for p in sigma[m]]
            # source rows of edge / chunks of h0 in XOR-position order
            row_perm = np.concatenate(
                [np.arange(TL) + p * TL for p in perm]
            )
            edge_m = np.ascontiguousarray(esh[m][row_perm, :]).astype(bf)
            chunk_perm = [2 * p + q for p in perm for q in (0, 1)]
            h0sb_m = np.ascontiguousarray(
                h0_chunks[chunk_perm].transpose(1, 0, 2).reshape(128, NCH * H)
            ).astype(bf)
        else:
            edge_m = np.ascontiguousarray(esh[m]).astype(bf)
            h0sb_m = np.ascontiguousarray(
                h0_chunks.transpose(1, 0, 2).reshape(128, NCH * H)
            ).astype(bf)
        in_maps.append(
            {
                "edge_in": edge_m,
                "h0sb_in": h0sb_m,
                "hid_in": h0.astype(bf),
                "hT0_in": hT0,
                "hT0b_in": hT0.astype(bf),
                "xT_in": xT.astype(bf),
                "w_in": w_flat,
                "win_in": win.astype(bf),
                "uzr_in": uzr.astype(bf),
                "uh_in": uh_.astype(bf),
                "bz_in": bz,
                "br_in": br,
                "bh_in": bh,
            }
        )
    return in_maps


_NC_CACHE: dict = {}


# The remote-DMA (SBUF-to-SBUF allgather) path is correct in MultiCoreSim and
# in isolated HW probes (any payload size/address, all 7 XOR slots, incl. all
# cross-die lanes concurrently), but in the full kernel the cross-die (D2D)
# transfers lose their data payloads on repeated fills (sem bumps arrive, data
# doesn't), which desc-gen pacing did not cure.  Until that HW interaction is
# understood, the proven NRT-collective path ships.
USE_RDMA = False


def _get_nc(
    iteration: int, reps: int = 1, ablate: frozenset = frozenset()
) -> bass.Bass:
    use_rdma = USE_RDMA and _get_sigma() is not False
    key = (iteration, reps, ablate, use_rdma)
    if key not in _NC_CACHE:
        _NC_CACHE[key] = build(iteration, reps=reps, ablate=ablate, use_rdma=use_rdma)
    return _NC_CACHE[key]


def kernel(
    input,
    hidden,
    edge_matrix,
    ba,
    wz_wr_wh,
    uz_ur,
    uh,
    input_wzrh,
    bw,
    iteration,
):
    iteration = int(iteration)
    if iteration <= 0:
        return np.asarray(hidden, np.float32).copy()

    sigma = _get_sigma() if USE_RDMA else False
    nc = _get_nc(iteration)
    in_maps = _host_prep(
        input, hidden, edge_matrix, ba, wz_wr_wh, uz_ur, uh, input_wzrh, bw,
        sigma=None if sigma is False else sigma,
    )
    res = run_bass_kernel_spmd(nc, in_maps, list(range(M)))
    out = np.concatenate([res.results[m]["h_out"] for m in range(M)], axis=0)
    return out[None]


# revision 31
# speedup vs baseline: 1.0141x; 1.0141x over previous
"""Trainium2 Bass kernel for GatedGraphXBias (gnn_message_passing), v3.

Reference math per iteration (T=2048 notes, E=12 edge types, H=64):
    act[e]  = edge[e].T @ h                      # [T, H]
    a       = sum_e (act[e] + ba[e]) @ W[e] + bw # [T, 3H] -> az|ar|ah
    a      += x @ Win                            # hoisted input projection
    z       = sigmoid(az + h @ Uz)
    r       = sigmoid(ar + h @ Ur)
    h~      = tanh(ah + (r*h) @ Uh)
    h       = (1-z)*h + z*h~

Sequence-parallel over T across 8 cores (TL=256 notes each).

v3: the per-iteration AllGather of h (NRT collective through DRAM + reload,
~20us/iter of serial latency) is replaced by direct SBUF->SBUF
remote_dma_broadcast pushes: each core sends its new h slice [128, 2H] bf16
to XOR-neighbor j (j=1..7) writing position j of the receiver's resident
gather buffer.  Descriptor generation (~1us/send on the Pool sequencer) is
hoisted into the mm1 window; a single trigger_dma fires all 7 sends the
moment hnew lands in SBUF.  Receivers gate their mm1 on a remote-bumped
semaphore (+2/send, 14/fill), double-buffered A/B by fill parity.  The
XOR-position permutation of source-note chunks is folded into the per-core
host-side layout of edge_in / h0sb_in, discovered at runtime by a tiny
probe kernel (sigma_m(j) = which core's slice lands at position j).  Falls
back to the v2 collective path if the probe fails.

v2 retained: fp16 matmul operands (f32 PSUM), mm1 column-group pairing via
tile_position (2 edge types concurrently in PE cols 0-63/64-127), mm2 over
the pair-stacked K=128, slot-major mm1 for it>=1 so PSUM copies + mm2
overlap the mm1 stream, r-first gate tail.
"""

import sys

sys.path.insert(0, "/opt/trn_rl_repo")

import numpy as np
import concourse.bass as bass
import concourse.mybir as mybir
import concourse.tile as tile
from concourse import library_config
from concourse.bass_utils import run_bass_kernel_spmd
from concourse.library_overlay import lower_extended_insts
from concourse.masks import make_identity
from concourse.vector_clock import ScopedClock

E, T, H, IN = 12, 2048, 64, 128
M = 8  # cores
TL = T // M  # 256 local notes per core
NCH = T // 128  # 16 contraction chunks of 128 source notes
NSLOT = 3  # mm1 PSUM slots, each covering 2 edge-type pairs
F32 = mybir.dt.float32
BF16 = mybir.dt.float16  # fp16: 11-bit mantissa (bf16's 8 was too coarse here)
U8 = mybir.dt.uint8
SIG = mybir.ActivationFunctionType.Sigmoid
TANH = mybir.ActivationFunctionType.Tanh
COPY = mybir.ActivationFunctionType.Copy
# per-chunk edge column order: slot k holds [e=4k (A), e=4k+2 (A), e=4k+1 (B), e=4k+3 (B)]
E_ORDER = [0, 2, 1, 3, 4, 6, 5, 7, 8, 10, 9, 11]


class SplitDrainTileContext(tile.TileContext):
    """TileContext that limits every instruction to a single sync wait.

    This walrus build rejects >1 sync wait command on an instruction
    (setupSyncWait: "Too many sync wait commands"), so extra waits are
    peeled onto standalone same-engine NoOps emitted just before the
    instruction — semantically identical (the engine stream waits
    sequentially at the same program point).

    Also supports `inject_wait(inst_name, sem, value)`: a sem-ge wait
    materialized as a NoOp at COMMIT time (post-scheduling), so waits that
    are satisfied by REMOTE cores (remote_dma sem bumps) never reach the
    single-core scheduling sim, which would otherwise deadlock on them."""

    def inject_wait(self, inst_name: str, sem, value: int):
        if not hasattr(self, "_inj_waits"):
            self._inj_waits = {}
        self._inj_waits.setdefault(inst_name, []).append((sem, value))

    def inject_inc_after(self, inst_name: str, sem, value: int):
        """Emit `sem += value` as an EventSemaphore right AFTER inst_name in
        its engine stream (commit-time; engine queues execute in order, so
        the inc fires once the instruction completes).  Used instead of
        then_inc: the race detector allows only one update per regular
        instruction."""
        if not hasattr(self, "_inj_incs"):
            self._inj_incs = {}
        self._inj_incs.setdefault(inst_name, []).append((sem, value))

    def _commit_instruction(self, inst, lazy_reg_writes: bool = True):
        for sem, value in getattr(self, "_inj_waits", {}).pop(inst.name, ()):
            w = mybir.SyncWait(
                sync_type="semaphore",
                id=sem.num,
                ant_name=sem.name,
                wait_mode="sem-ge-imm",
                wait_value=value,
                wait_reg=None,
            )
            nop = mybir.InstNoOp(
                name=f"injwait-{self.nc.next_id()}",
                sync_info=mybir.SyncInfo(on_wait=[w], on_update=[]),
                bass_nofuse=True,
                engine=inst.engine,
            )
            super()._commit_instruction(nop, lazy_reg_writes=False)
        si = getattr(inst, "sync_info", None)
        if si is not None and len(si.on_wait) > 1:
            waits = list(si.on_wait)
            inst.sync_info = mybir.SyncInfo(
                on_wait=[waits[-1]], on_update=list(si.on_update)
            )
            for w in waits[:-1]:
                nop = mybir.InstNoOp(
                    name=f"splitwait-{self.nc.next_id()}",
                    sync_info=mybir.SyncInfo(on_wait=[w], on_update=[]),
                    bass_nofuse=True,
                    engine=inst.engine,
                )
                super()._commit_instruction(nop, lazy_reg_writes=False)
        super()._commit_instruction(inst, lazy_reg_writes)
        # running post-completion totals of every sem, in committed order
        if not hasattr(self, "_sem_totals"):
            self._sem_totals = {}
        si2 = getattr(inst, "sync_info", None)
        if si2 is not None:
            for u in si2.on_update:
                if u.update_mode == "sem-add-imm":
                    inc = u.update_value
                elif u.update_mode == "sem-inc":
                    inc = u.update_value if u.update_value else 1
                else:
                    continue
                self._sem_totals[u.id] = self._sem_totals.get(u.id, 0) + inc
        for sem, value in getattr(self, "_inj_incs", {}).pop(inst.name, ()):
            # The inc must fire only after `inst`'s DATA completes, not when
            # the sequencer reaches it — wait on inst's own completion sem
            # at its running total.
            waits = []
            if si2 is not None and si2.on_update:
                u0 = si2.on_update[0]
                waits = [
                    mybir.SyncWait(
                        sync_type="semaphore",
                        id=u0.id,
                        ant_name=u0.ant_name,
                        wait_mode="sem-ge-imm",
                        wait_value=self._sem_totals[u0.id],
                        wait_reg=None,
                    )
                ]
            u = mybir.SyncUpdate(
                sync_type="semaphore",
                id=sem.num,
                ant_name=sem.name,
                update_mode="sem-add-imm",
                update_value=value,
                update_reg=None,
            )
            ev = mybir.InstEventSemaphore(
                name=f"injinc-{self.nc.next_id()}",
                sync_info=mybir.SyncInfo(on_wait=waits, on_update=[u]),
                bass_nofuse=True,
                engine=inst.engine,
            )
            super()._commit_instruction(ev, lazy_reg_writes=False)
            self._sem_totals[sem.num] = self._sem_totals.get(sem.num, 0) + value

    def _drain_and_barrier(self, tick_clock, wait_clock):
        drain_inst = self.nc.sync.drain()
        wait_clock.add_sem_waits(
            drain_inst.ins, ScopedClock({None: tick_clock.global_clock})
        )
        si = drain_inst.ins.sync_info
        waits = list(si.on_wait) if si is not None else []
        upds = list(si.on_update) if si is not None else []
        if len(waits) > 1:
            drain_inst.ins.sync_info = mybir.SyncInfo(on_wait=waits[:1], on_update=upds)
            for w in waits[1:]:
                nop = self.nc.sync.nop(nofuse=True, hint="split_drain_waits")
                nop.ins.sync_info = mybir.SyncInfo(on_wait=[w], on_update=[])

        self.nc.all_engine_barrier()
        assert self.sems is not None
        popped = self.nc._tile_sem_poison_stack.pop()
        assert popped is self._sem_poison
        self.nc.clear_and_free_semaphores(list(self.sems.allocated().values()))
        self.nc.all_engine_barrier()


# ---------------------------------------------------------------------------
# runtime probe: discover sigma_m(j) = jax core whose broadcast slot j lands
# at position j of receiver m (XOR topology over the physical tpb ids)
# ---------------------------------------------------------------------------


def build_probe() -> bass.Bass:
    nc = bass.Bass(
        "TRN2",
        target_bir_lowering=False,
        debug=False,
        num_devices=M,
        dynamic_dma_scratch_size=16384,
    )
    val_in = nc.declare_dram_parameter("val_in", [128, 8], F32, isOutput=False)
    gath_out = nc.declare_dram_parameter("gath_out", [128, 8 * M], F32, isOutput=True)

    with SplitDrainTileContext(nc) as tc:
        with (
            tc.tile_pool(name="work", bufs=1) as wpool,
            tc.tile_pool(name="dram", bufs=1, space="DRAM") as dpool,
        ):
            rsem = nc.alloc_semaphore("probe_rsem")
            lsem = nc.alloc_semaphore("probe_lsem")
            isem = nc.alloc_semaphore("probe_isem")

            val_sb = wpool.tile([128, 8], F32, name="val")
            nc.sync.dma_start(out=val_sb[:], in_=val_in[:])
            gbuf = wpool.tile([128, 8 * M], F32, name="gbuf")
            nc.vector.tensor_copy(gbuf[:, 0:8], val_sb[:])
            nc.vector.sem_inc(isem, 1)

            bar_in = dpool.tile([1, 1], U8, name="bar_in")
            bar_out = dpool.tile([M, 1], U8, name="bar_out", addr_space="Shared")
            nc.gpsimd.wait_ge(isem, 1)
            nc.gpsimd.collective_compute(
                "AllGather",
                mybir.AluOpType.bypass,
                replica_groups=[list(range(M))],
                ins=[bar_in[:]],
                outs=[bar_out[:]],
            )
            bar_sb = wpool.tile([1, 1], U8, name="bar_sb")
            junk_sb = wpool.tile([1, 1], U8, name="junk_sb")
            nc.gpsimd.dma_start(out=bar_sb[:], in_=bar_out[0:1, :])
            nc.gpsimd.tensor_copy(junk_sb[:], bar_sb[:])

            nc.gpsimd.load_library(library_config.remote_dma)
            for j in range(1, M):
                rdests: list = [None] * M
                rdests[j] = (0, j)
                nc.gpsimd.remote_dma_broadcast(
                    out_ap=gbuf[:, j * 8 : (j + 1) * 8],
                    in_ap=val_sb[:],
                    remote_sem=rsem,
                    local_sem=lsem,
                    rdests=rdests,
                )
            nc.gpsimd.trigger_dma(count=None)

            st = nc.sync.dma_start(out=gath_out[:], in_=gbuf[:])
            tc.inject_wait(st.ins.name, rsem, 14)

    lower_extended_insts(nc)
    return nc


_SIGMA: "np.ndarray | None | bool" = None  # None=unknown, False=probe failed


def _get_sigma():
    global _SIGMA
    if _SIGMA is None:
        try:
            nc = build_probe()
            in_maps = [
                {"val_in": np.full((128, 8), float(m), np.float32)} for m in range(M)
            ]
            res = run_bass_kernel_spmd(nc, in_maps, list(range(M)))
            sig = np.zeros((M, M), int)
            for m in range(M):
                out = res.results[m]["gath_out"]
                for j in range(M):
                    blk = out[:, j * 8 : (j + 1) * 8]
                    v = blk[0, 0]
                    if not np.all(blk == v):
                        raise ValueError(f"non-uniform probe block m={m} j={j}")
                    sig[m, j] = int(v)
            for m in range(M):
                if sorted(sig[m]) != list(range(M)) or sig[m, 0] != m:
                    raise ValueError(f"bad probe row {m}: {sig[m]}")
                for j in range(M):
                    if sig[sig[m, j], j] != m:
                        raise ValueError(f"asymmetric probe at m={m} j={j}")
            _SIGMA = sig
        except Exception as e:  # noqa: BLE001
            print(f"kernel: rdma probe failed ({e!r}); falling back to collective")
            _SIGMA = False
    return _SIGMA


# ---------------------------------------------------------------------------
# main kernel
# ---------------------------------------------------------------------------


def build(
    iteration: int,
    reps: int = 1,
    ablate: frozenset = frozenset(),
    use_rdma: bool = True,
    debug_bufs: bool = False,
) -> bass.Bass:
    nc = bass.Bass(
        "TRN2",
        target_bir_lowering=False,
        debug=False,
        num_devices=M,
        dynamic_dma_scratch_size=32768 if use_rdma else 2048,
        num_swdge_queues=2 if use_rdma else 1,
    )

    # Per-core inputs (host pre-arranged; in rdma mode edge_in/h0sb_in source
    # chunks are permuted per-core into XOR-position order by sigma):
    #   edge_in : [T, E*TL] fp16, row = source note, col = chunk-local
    #             [slotk: eA0 | eA1 | eB0 | eB1] blocks of TL (see E_ORDER)
    #   h0sb_in : [128, NCH*H] fp16  initial h in chunk-major SBUF layout
    #   hid_in  : [T, H] fp16        initial h, note-major (coll-ablate path)
    #   hT0_in  : [H, TL] f32        local initial hidden, transposed
    #   hT0b_in : [H, TL] fp16
    #   xT_in   : [IN, TL] fp16      local input features, transposed
    #   w_in    : [128, 6*3*H] fp16  pair-stacked W: col p*192+g*64+j is
    #             gate g of pair p; rows 0-63 = W[2p], 64-127 = W[2p+1]
    #   win_in  : [IN, 3H] fp16
    #   uzr_in  : [H, 2H] fp16
    #   uh_in   : [H, H] fp16
    #   bz/br/bh: [H, 1] f32         folded biases (bw + sum_e ba[e] @ W[e])
    edge_in = nc.declare_dram_parameter("edge_in", [T, E * TL], BF16, isOutput=False)
    h0sb_in = nc.declare_dram_parameter("h0sb_in", [128, NCH * H], BF16, isOutput=False)
    hid_in = nc.declare_dram_parameter("hid_in", [T, H], BF16, isOutput=False)
    hT0_in = nc.declare_dram_parameter("hT0_in", [H, TL], F32, isOutput=False)
    hT0b_in = nc.declare_dram_parameter("hT0b_in", [H, TL], BF16, isOutput=False)
    xT_in = nc.declare_dram_parameter("xT_in", [IN, TL], BF16, isOutput=False)
    w_in = nc.declare_dram_parameter("w_in", [128, 6 * 3 * H], BF16, isOutput=False)
    win_in = nc.declare_dram_parameter("win_in", [IN, 3 * H], BF16, isOutput=False)
    uzr_in = nc.declare_dram_parameter("uzr_in", [H, 2 * H], BF16, isOutput=False)
    uh_in = nc.declare_dram_parameter("uh_in", [H, H], BF16, isOutput=False)
    bz_in = nc.declare_dram_parameter("bz_in", [H, 1], F32, isOutput=False)
    br_in = nc.declare_dram_parameter("br_in", [H, 1], F32, isOutput=False)
    bh_in = nc.declare_dram_parameter("bh_in", [H, 1], F32, isOutput=False)
    h_out = nc.declare_dram_parameter("h_out", [TL, H], F32, isOutput=True)
    if debug_bufs:
        dbg_outs = {
            n: nc.declare_dram_parameter(n, [128, NCH * H], BF16, isOutput=True)
            for n in ("hbuf0_out", "hbuf1_out")
        }
        dbg_outs.update(
            {
                n: nc.declare_dram_parameter(n, [128, 2 * H], BF16, isOutput=True)
                for n in ("hnewb0_out", "hnewb1_out")
            }
        )

    # (sem, value, [remote matmul inst names]) per fill — validated post-lower
    fill_checks: list = []
    pool_checks: dict = {"lib": None, "bar_copy": None, "first_trig": None,
                         "preps": [], "trigs": []}

    with SplitDrainTileContext(nc) as tc:
        with (
            # comm pool FIRST: remote-DMA-addressed tiles must sit at low
            # SBUF offsets (the D2D cross-die desc path mishandles high
            # destination offsets; intra-die is fine)
            tc.tile_pool(name="comm", bufs=1) as commpool,
            tc.tile_pool(name="edge", bufs=1) as edge_pool,
            tc.tile_pool(name="const", bufs=1) as cpool,
            tc.tile_pool(name="work", bufs=1) as wpool,
            tc.tile_pool(name="psum", bufs=1, space="PSUM") as ppool,
            tc.tile_pool(name="dram", bufs=2, space="DRAM") as dpool,
        ):
            if use_rdma:
                # persistent gather buffers (remote-written, parity by fill)
                hbuf = [
                    commpool.tile([128, NCH * H], BF16, name=f"hbuf{p}")
                    for p in range(2)
                ]
                # persistent broadcast sources (parity by fill)
                hnewb = [
                    commpool.tile([128, 2 * H], BF16, name=f"hnewb{p}")
                    for p in range(2)
                ]
            # ---- constants / weights (loaded once) ----
            w_sb = cpool.tile([128, 6 * 3 * H], BF16)
            nc.sync.dma_start(out=w_sb[:], in_=w_in[:])
            uzr_sb = cpool.tile([H, 2 * H], BF16)
            nc.sync.dma_start(out=uzr_sb[:], in_=uzr_in[:])
            uh_sb = cpool.tile([H, H], BF16)
            nc.sync.dma_start(out=uh_sb[:], in_=uh_in[:])
            win_sb = cpool.tile([IN, 3 * H], BF16)
            nc.sync.dma_start(out=win_sb[:], in_=win_in[:])
            xT_sb = cpool.tile([IN, TL], BF16)
            nc.sync.dma_start(out=xT_sb[:], in_=xT_in[:])
            bz_sb = cpool.tile([H, 1], F32)
            nc.sync.dma_start(out=bz_sb[:], in_=bz_in[:])
            br_sb = cpool.tile([H, 1], F32)
            nc.sync.dma_start(out=br_sb[:], in_=br_in[:])
            bh_sb = cpool.tile([H, 1], F32)
            nc.sync.dma_start(out=bh_sb[:], in_=bh_in[:])
            id64 = cpool.tile([H, H], F32)
            make_identity(nc, id64[:])
            id64b = cpool.tile([H, H], BF16)
            make_identity(nc, id64b[:])

            if use_rdma:
                rsem = [nc.alloc_semaphore("rsemA"), nc.alloc_semaphore("rsemB")]
                # rotated local sems (SWDGE ring reclaim: "rotate sems")
                lsem = [nc.alloc_semaphore("lsemA"), nc.alloc_semaphore("lsemB")]
                hsem = nc.alloc_semaphore("hsem")
                # persistent gather buffers (remote-written, parity by fill)
                hbuf = [
                    cpool.tile([128, NCH * H], BF16, name=f"hbuf{p}") for p in range(2)
                ]
                # persistent broadcast sources (parity by fill)
                hnewb = [
                    cpool.tile([128, 2 * H], BF16, name=f"hnewb{p}") for p in range(2)
                ]
                # entry barrier: peers must be in-kernel before remote writes
                bar_in = dpool.tile([1, 1], U8, name="bar_in", tag="bar_in", bufs=1)
                bar_out = dpool.tile(
                    [M, 1], U8, name="bar_out", tag="bar_out", bufs=1,
                    addr_space="Shared",
                )
                nc.gpsimd.collective_compute(
                    "AllGather",
                    mybir.AluOpType.bypass,
                    replica_groups=[list(range(M))],
                    ins=[bar_in[:]],
                    outs=[bar_out[:]],
                )
                bar_sb = cpool.tile([1, 1], U8, name="bar_sb")
                junk_sb = cpool.tile([1, 1], U8, name="junk_sb")
                nc.gpsimd.dma_start(out=bar_sb[:], in_=bar_out[0:1, :])
                bcp = nc.gpsimd.tensor_copy(junk_sb[:], bar_sb[:])
                barsem = nc.alloc_semaphore("barsem")
                tc.inject_inc_after(bcp.ins.name, barsem, 1)
                pool_checks["bar_copy"] = bcp.ins.name
                # Tile-visible edge barrier-chain -> preps: the preps read
                # hnewb, so a bar_sb-sourced write into each hnewb forces the
                # scheduler to place the whole chain before the first fill's
                # desc-gen (and thus before the first trigger, which would
                # otherwise stall the Pool queue on barsem ahead of the cc).
                for p in range(2):
                    nc.gpsimd.tensor_copy(hnewb[p][0:1, 0:1], bar_sb[:])
                lib = nc.gpsimd.load_library(library_config.remote_dma)
                pool_checks["lib"] = lib.ins.name
                fills = [0, 0]  # fills into parity A/B so far
                fill_no = 0  # global fill counter (across reps)
                hsem_target = 0

            for rep in range(reps):
                # ---- h state first (so iteration 0 isn't queued behind the
                # edge load on the SP HWDGE ring) ----
                if use_rdma:
                    h0_sb = cpool.tile(
                        [128, NCH * H], BF16, name="h0sb"
                    ) if rep == 0 else h0_sb
                    nc.sync.dma_start(out=h0_sb[:], in_=h0sb_in[:])
                else:
                    h_sb = wpool.tile([128, NCH * H], BF16, name="h", tag="h", bufs=1)
                    nc.sync.dma_start(out=h_sb[:], in_=h0sb_in[:])
                hT_sb = wpool.tile([H, TL], F32, name="hT", tag="hT", bufs=2)
                nc.sync.dma_start(out=hT_sb[:], in_=hT0_in[:])
                hTb_sb = wpool.tile([H, TL], BF16, name="hTb", tag="hTb", bufs=2)
                nc.sync.dma_start(out=hTb_sb[:], in_=hT0b_in[:])

                # ---- resident edge shard: 16 chunk DMAs, alternating the
                # SP / ACT HWDGE rings so chunk c lands ~c*1.2us in and the
                # chunk-outer mm1 of iteration 0 streams right behind ----
                edge_sb = []
                for c in range(NCH):
                    et = edge_pool.tile(
                        [128, E * TL], BF16, name=f"edge_c{c}", tag=f"edge_c{c}"
                    )
                    edge_sb.append(et)
                for c in range(NCH):
                    eng = nc.sync if c % 2 == 0 else nc.scalar
                    eng.dma_start(
                        out=edge_sb[c][:],
                        in_=edge_in[c * 128 : (c + 1) * 128, :],
                    )

                for it in range(iteration):
                    last = it == iteration - 1
                    if use_rdma:
                        hsrc = h0_sb if it == 0 else hbuf[(fill_no - 1) % 2]
                    else:
                        hsrc = h_sb

                    # Gate pre-activation groups [H, TL]: folded input
                    # projection starts each, U-gate matmuls fold in, then
                    # the 18 pair-stacked mm2 matmuls accumulate.
                    az_ps = ppool.tile([H, TL], F32, tag="az", bufs=1)
                    ar_ps = ppool.tile([H, TL], F32, tag="ar", bufs=1)
                    ah_ps = ppool.tile([H, TL], F32, tag="ah", bufs=1)
                    for g, ps in enumerate((az_ps, ar_ps, ah_ps)):
                        nc.tensor.matmul(
                            ps[:],
                            lhsT=win_sb[:, g * H : (g + 1) * H],
                            rhs=xT_sb[:],
                            start=True,
                            stop=False,
                            skip_group_check=True,
                        )
                    for g, ps in enumerate((az_ps, ar_ps)):
                        nc.tensor.matmul(
                            ps[:],
                            lhsT=uzr_sb[:, g * H : (g + 1) * H],
                            rhs=hTb_sb[:],
                            start=False,
                            stop="mm2" in ablate or "mm1" in ablate,
                            skip_group_check=True,
                        )

                    # -- mm1: col-group tiled over e-pairs --
                    act_ps = []
                    for k in range(NSLOT):
                        act_k = ppool.tile(
                            [128, 2 * TL], F32, name=f"act{k}", tag=f"act{k}", bufs=1
                        )
                        act_ps.append(act_k)
                    if "mm1" not in ablate:
                        # iteration 0: chunk-outer (pipelines behind the edge
                        # load); later iterations: slot-outer, so slot 0's
                        # PSUM copy + mm2 overlap the remaining mm1 stream
                        if it == 0:
                            order = [(c, k) for c in range(NCH) for k in range(NSLOT)]
                        else:
                            order = [(c, k) for k in range(NSLOT) for c in range(NCH)]
                        remote_mms: list = []
                        wait_spec = None
                        if use_rdma and it > 0 and "comm" not in ablate:
                            p_read = (fill_no - 1) % 2
                            wait_spec = (rsem[p_read], 14 * fills[p_read])
                        for c, k in order:
                            lhs = hsrc[:, c * H : (c + 1) * H]
                            base = k * 4 * TL
                            mm_a = nc.tensor.matmul(
                                act_ps[k][0:H, :],
                                lhsT=lhs,
                                rhs=edge_sb[c][:, base : base + 2 * TL],
                                start=(c == 0),
                                stop=(c == NCH - 1),
                                tile_position=(0, 0),
                                skip_group_check=True,
                            )
                            mm_b = nc.tensor.matmul(
                                act_ps[k][H:128, :],
                                lhsT=lhs,
                                rhs=edge_sb[c][:, base + 2 * TL : base + 4 * TL],
                                start=(c == 0),
                                stop=(c == NCH - 1),
                                tile_position=(0, H),
                                skip_group_check=True,
                            )
                            if wait_spec is not None and c >= 2:
                                if not remote_mms:
                                    tc.inject_wait(
                                        mm_a.ins.name, wait_spec[0], wait_spec[1]
                                    )
                                remote_mms.append(mm_a.ins.name)
                                remote_mms.append(mm_b.ins.name)
                        if wait_spec is not None:
                            fill_checks.append(
                                (wait_spec[0], wait_spec[1], remote_mms)
                            )

                        # -- rdma: desc-gen for this fill's 7 sends, hidden
                        # under the mm1 window (Pool is otherwise idle) --
                        if use_rdma and not last and "comm" not in ablate:
                            p_fill = fill_no % 2
                            for j in range(1, M):
                                rdests: list = [None] * M
                                rdests[j] = (0, j)
                                pr = nc.gpsimd.remote_dma_broadcast(
                                    out_ap=hbuf[p_fill][:, 2 * j * H : (2 * j + 2) * H],
                                    in_ap=hnewb[p_fill][:],
                                    remote_sem=rsem[p_fill],
                                    local_sem=lsem[p_fill],
                                    rdests=rdests,
                                    queue_num=p_fill,
                                )
                                if j == 1 and fill_no > 0:
                                    # pace desc-gen: my receipt of fill G-1
                                    # (rsem) proves every peer's mm1 at
                                    # iteration G-1 ran, i.e. peers consumed
                                    # my fill G-2 sends -> those SWDGE ring
                                    # entries are reclaimable.  Fill G-1's
                                    # entries are protected by ring capacity
                                    # (>= 2 fills at 32KB scratch).
                                    pp = (fill_no - 1) % 2
                                    tc.inject_wait(
                                        pr.ins.name, rsem[pp], 14 * fills[pp]
                                    )
                                pool_checks["preps"].append(pr.ins.name)

                        # -- act PSUM -> SBUF (bf16) + mm2 --
                        for k in range(NSLOT):
                            act_sb = wpool.tile(
                                [128, 2 * TL], BF16, tag=f"actsb{k}", bufs=2
                            )
                            if k == 1:
                                nc.scalar.activation(
                                    act_sb[:],
                                    act_ps[k][:],
                                    COPY,
                                )
                            else:
                                nc.vector.tensor_copy(act_sb[:], act_ps[k][:])
                            if "mm2" in ablate:
                                continue
                            for j in range(2):
                                p = 2 * k + j
                                rhs = act_sb[:, j * TL : (j + 1) * TL]
                                for g, ps in enumerate((az_ps, ar_ps, ah_ps)):
                                    nc.tensor.matmul(
                                        ps[:],
                                        lhsT=w_sb[
                                            :,
                                            p * 3 * H + g * H : p * 3 * H + (g + 1) * H,
                                        ],
                                        rhs=rhs,
                                        start=False,
                                        stop=(p == 5 and g != 2),
                                        skip_group_check=True,
                                    )
                    # -- gates --  (r first: it heads the rh -> Uh -> tanh
                    # critical chain; z's products are precomputed in its
                    # shadow so only 2 DVE ops remain after the tanh)
                    r_sb = wpool.tile([H, TL], F32, tag="r")
                    nc.scalar.activation(r_sb[:], ar_ps[:], SIG, bias=br_sb[:])
                    z_sb = wpool.tile([H, TL], F32, tag="z")
                    nc.scalar.activation(z_sb[:], az_ps[:], SIG, bias=bz_sb[:])
                    rh_sb = wpool.tile([H, TL], BF16, tag="rh")
                    nc.vector.tensor_mul(rh_sb[:], r_sb[:], hT_sb[:])
                    nc.tensor.matmul(
                        ah_ps[:],
                        lhsT=uh_sb[:],
                        rhs=rh_sb[:],
                        start=False,
                        stop=True,
                        skip_group_check=True,
                    )
                    omz_sb = wpool.tile([H, TL], F32, tag="omz")
                    nc.vector.tensor_scalar(
                        omz_sb[:], z_sb[:], -1.0, 1.0,
                        mybir.AluOpType.mult, mybir.AluOpType.add,
                    )  # 1-z
                    omzh_sb = wpool.tile([H, TL], F32, tag="omzh")
                    nc.vector.tensor_mul(omzh_sb[:], omz_sb[:], hT_sb[:])  # (1-z)*h
                    ht_sb = wpool.tile([H, TL], F32, tag="ht")
                    nc.scalar.activation(ht_sb[:], ah_ps[:], TANH, bias=bh_sb[:])

                    zd_sb = wpool.tile([H, TL], F32, tag="zd")
                    nc.vector.tensor_mul(zd_sb[:], z_sb[:], ht_sb[:])  # z*h~
                    hnewT_sb = wpool.tile([H, TL], F32, tag="hT", bufs=2)
                    nc.vector.tensor_add(hnewT_sb[:], zd_sb[:], omzh_sb[:])

                    if last:
                        # transpose f32 -> [TL, H] and store the output
                        hnew_sb = wpool.tile([128, 2 * H], F32, tag="hnew")
                        for half in range(2):
                            tr_ps = ppool.tile([128, H], F32, tag="tr")
                            nc.tensor.transpose(
                                tr_ps[:],
                                hnewT_sb[:, half * 128 : (half + 1) * 128],
                                id64[:],
                            )
                            nc.vector.tensor_copy(
                                hnew_sb[:, half * H : (half + 1) * H], tr_ps[:]
                            )
                        nc.sync.dma_start(
                            out=h_out[:].rearrange("(c p) j -> p c j", p=128),
                            in_=hnew_sb[:].rearrange("p (c j) -> p c j", c=2),
                        )
                    elif use_rdma:
                        # fp16 hT copy for the next iteration's U-gate matmul
                        hnTb_sb = wpool.tile([H, TL], BF16, tag="hTb", bufs=2)
                        nc.vector.tensor_copy(hnTb_sb[:], hnewT_sb[:])
                        p_fill = fill_no % 2
                        for half in range(2):
                            tr_ps = ppool.tile([128, H], F32, tag="tr")
                            nc.tensor.transpose(
                                tr_ps[:],
                                hnewT_sb[:, half * 128 : (half + 1) * 128],
                                id64[:],
                            )
                            if "comm" not in ablate:
                                cpv = nc.vector.tensor_copy(
                                    hnewb[p_fill][:, half * H : (half + 1) * H],
                                    tr_ps[:],
                                )
                                tc.inject_inc_after(cpv.ins.name, hsem, 1)
                            # self slice -> position 0 of the gather buffer
                            nc.scalar.activation(
                                hbuf[p_fill][:, half * H : (half + 1) * H],
                                tr_ps[:],
                                COPY,
                            )
                        if "comm" not in ablate:
                            hsem_target += 2
                            trig = nc.gpsimd.trigger_dma(
                                count=None, queue_num=fill_no % 2
                            )
                            tc.inject_wait(trig.ins.name, hsem, hsem_target)
                            pool_checks["trigs"].append(trig.ins.name)
                            if pool_checks["first_trig"] is None:
                                pool_checks["first_trig"] = trig.ins.name
                                # peers must be in-kernel before the first
                                # remote write actually fires
                                tc.inject_wait(trig.ins.name, barsem, 1)
                            fills[p_fill] += 1
                            fill_no += 1
                        hT_sb = hnewT_sb
                        hTb_sb = hnTb_sb
                    else:
                        # v2 collective path
                        hnTb_sb = wpool.tile([H, TL], BF16, tag="hTb", bufs=2)
                        nc.vector.tensor_copy(hnTb_sb[:], hnewT_sb[:])
                        hnew_sb = wpool.tile([128, 2 * H], BF16, tag="hnewb")
                        for half in range(2):
                            tr_ps = ppool.tile([128, H], F32, tag="tr")
                            nc.tensor.transpose(
                                tr_ps[:],
                                hnewT_sb[:, half * 128 : (half + 1) * 128],
                                id64[:],
                            )
                            nc.vector.tensor_copy(
                                hnew_sb[:, half * H : (half + 1) * H], tr_ps[:]
                            )

                        if "coll" not in ablate:
                            ag_in = dpool.tile([TL, H], BF16, tag="ag_in")
                            for half in range(2):
                                eng = nc.sync if half == 0 else nc.scalar
                                eng.dma_start(
                                    out=ag_in[half * 128 : (half + 1) * 128, :],
                                    in_=hnew_sb[:, half * H : (half + 1) * H],
                                )
                            ag_out = dpool.tile(
                                [T, H], BF16, tag="ag_out", addr_space="Shared"
                            )
                            nc.gpsimd.collective_compute(
                                "AllGather",
                                mybir.AluOpType.bypass,
                                replica_groups=[list(range(M))],
                                ins=[ag_in[:]],
                                outs=[ag_out[:]],
                            )
                            gather_src = ag_out
                            warm_ps = ppool.tile(
                                [H, 2 * TL], F32, name="warm", tag="warm", bufs=1
                            )
                            for _ in range(24):
                                nc.tensor.matmul(
                                    warm_ps[:],
                                    lhsT=hnew_sb[:, 0:H],
                                    rhs=edge_sb[0][:, 0 : 2 * TL],
                                    start=True,
                                    stop=True,
                                    skip_group_check=True,
                                )
                        else:
                            gather_src = hid_in
                        if "hreload" not in ablate:
                            h_sb = wpool.tile([128, NCH * H], BF16, tag="h", bufs=1)
                            qch = NCH // 4
                            for qq in range(4):
                                eng = nc.sync if qq % 2 == 0 else nc.scalar
                                eng.dma_start(
                                    out=h_sb[
                                        :, qq * qch * H : (qq + 1) * qch * H
                                    ].rearrange("p (c j) -> p c j", c=qch),
                                    in_=gather_src[
                                        qq * qch * 128 : (qq + 1) * qch * 128, :
                                    ].rearrange("(c p) j -> p c j", p=128),
                                )
                        hT_sb = hnewT_sb
                        hTb_sb = hnTb_sb

            if use_rdma and debug_bufs:
                nc.sync.dma_start(out=dbg_outs["hbuf0_out"][:], in_=hbuf[0][:])
                nc.sync.dma_start(out=dbg_outs["hbuf1_out"][:], in_=hbuf[1][:])
                nc.sync.dma_start(out=dbg_outs["hnewb0_out"][:], in_=hnewb[0][:])
                nc.sync.dma_start(out=dbg_outs["hnewb1_out"][:], in_=hnewb[1][:])

    if use_rdma:
        _validate_stream_order(nc, fill_checks, pool_checks)
        lower_extended_insts(nc)
    return nc


def _validate_stream_order(nc, fill_checks, pool_checks):
    """The TileScheduler is free to reorder same-engine instructions; verify
    the final PE stream places each fill's injected rsem wait before every
    matmul that reads remote gather chunks, and the Pool stream places the
    library load / barrier chain before the first prep / trigger."""
    streams: dict = {}
    for fn in nc.m.functions:
        for bb in fn.blocks:
            for ins in bb.instructions:
                eng = getattr(ins, "engine", None)
                if eng is None:
                    continue
                streams.setdefault(eng, []).append(ins)
    pe = streams.get(mybir.EngineType.PE, [])
    pe_pos = {ins.name: i for i, ins in enumerate(pe)}
    for sem, value, mm_names in fill_checks:
        wpos = None
        for i, ins in enumerate(pe):
            si = getattr(ins, "sync_info", None)
            if si is None:
                continue
            for w in si.on_wait:
                if w.id == sem.num and w.wait_value == value:
                    wpos = i
                    break
            if wpos is not None:
                break
        assert wpos is not None, f"missing injected wait {sem.name}>={value}"
        for n in mm_names:
            assert pe_pos[n] > wpos, (
                f"matmul {n} (pos {pe_pos[n]}) precedes its rsem wait "
                f"{sem.name}>={value} (pos {wpos})"
            )
    pool = streams.get(mybir.EngineType.Pool, [])
    pool_pos = {ins.name: i for i, ins in enumerate(pool)}
    lib = pool_pos[pool_checks["lib"]]
    for pr in pool_checks["preps"]:
        assert pool_pos[pr] > lib, f"prep {pr} precedes library load"
    if pool_checks["first_trig"] is not None:
        # the barsem injected wait must sit directly before the first trigger
        ft = pool_pos[pool_checks["first_trig"]]
        has_barwait = any(
            w.ant_name == "barsem" and w.wait_value == 1
            for ins in pool[:ft]
            if getattr(ins, "sync_info", None) is not None
            for w in ins.sync_info.on_wait
        )
        assert has_barwait, "first trigger lacks preceding barsem wait"
    # triggers must follow their preps (7 per fill, FIFO)
    for i, tr in enumerate(pool_checks["trigs"]):
        for pr in pool_checks["preps"][7 * i : 7 * (i + 1)]:
            assert pool_pos[pr] < pool_pos[tr], f"prep {pr} after trigger {tr}"


def _host_prep(
    input, hidden, edge_matrix, ba, wz_wr_wh, uz_ur, uh, input_wzrh, bw, sigma="auto"
):
    """Pre-arrange full inputs into the per-core DMA layouts."""
    if isinstance(sigma, str) and sigma == "auto":
        sigma = _SIGMA if isinstance(_SIGMA, np.ndarray) else None
    bf = mybir.dt.np(BF16)
    x = np.asarray(input, np.float32)[0]  # [T, IN]
    h0 = np.ascontiguousarray(np.asarray(hidden, np.float32)[0])  # [T, H]
    edge = np.asarray(edge_matrix, np.float32)  # [E, T, T]
    ba = np.asarray(ba, np.float32)
    W = np.asarray(wz_wr_wh, np.float32)  # [E, H, 3H]
    uzr = np.ascontiguousarray(np.asarray(uz_ur, np.float32))
    uh_ = np.ascontiguousarray(np.asarray(uh, np.float32))
    win = np.ascontiguousarray(np.asarray(input_wzrh, np.float32))
    bw = np.asarray(bw, np.float32)

    # folded bias: bw + sum_e ba[e] @ W[e]
    btot = bw + np.einsum("eh,ehk->k", ba, W)  # [3H]
    bz = np.ascontiguousarray(btot[:H].reshape(H, 1))
    br = np.ascontiguousarray(btot[H : 2 * H].reshape(H, 1))
    bh = np.ascontiguousarray(btot[2 * H :].reshape(H, 1))

    # edge shards: per core m, [T, E*TL] with columns in E_ORDER blocks
    # esh[m][s, i*TL + tl] = edge[E_ORDER[i], s, m*TL + tl]
    eperm = edge[E_ORDER]  # [E, T, T]
    esh = np.ascontiguousarray(
        eperm.reshape(E, T, M, TL).transpose(2, 1, 0, 3)
    ).reshape(M, T, E * TL)

    # pair-stacked W: w_flat[row, p*192 + g*64 + j]
    #   rows 0-63 = W[2p][:, g*64+j], rows 64-127 = W[2p+1][:, g*64+j]
    w_pair = np.empty((128, 6, 3 * H), np.float32)
    for p in range(6):
        w_pair[:H, p] = W[2 * p]
        w_pair[H:, p] = W[2 * p + 1]
    w_flat = np.ascontiguousarray(w_pair.reshape(128, 6 * 3 * H)).astype(bf)

    h0_chunks = h0.reshape(NCH, 128, H)  # global chunk-major

    in_maps = []
    for m in range(M):
        xT = np.ascontiguousarray(x[m * TL : (m + 1) * TL, :].T)
        hT0 = np.ascontiguousarray(h0[m * TL : (m + 1) * TL, :].T)
        if sigma is not None:
            perm = [int(p) for p in sigma[m]]
            # source rows of edge / chunks of h0 in XOR-position order
            row_perm = np.concatenate(
                [np.arange(TL) + p * TL for p in perm]
            )
            edge_m = np.ascontiguousarray(esh[m][row_perm, :]).astype(bf)
            chunk_perm = [2 * p + q for p in perm for q in (0, 1)]
            h0sb_m = np.ascontiguousarray(
                h0_chunks[chunk_perm].transpose(1, 0, 2).reshape(128, NCH * H)
            ).astype(bf)
        else:
            edge_m = np.ascontiguousarray(esh[m]).astype(bf)
            h0sb_m = np.ascontiguousarray(
                h0_chunks.transpose(1, 0, 2).reshape(128, NCH * H)
            ).astype(bf)
        in_maps.append(
            {
                "edge_in": edge_m,
                "h0sb_in": h0sb_m,
                "hid_in": h0.astype(bf),
                "hT0_in": hT0,
                "hT0b_in": hT0.astype(bf),
                "xT_in": xT.astype(bf),
                "w_in": w_flat,
                "win_in": win.astype(bf),
                "uzr_in": uzr.astype(bf),
                "uh_in": uh_.astype(bf),
                "bz_in": bz,
                "br_in": br,
                "bh_in": bh,
            }
        )
    return in_maps


_NC_CACHE: dict = {}


# The remote-DMA (SBUF-to-SBUF allgather) path is correct in MultiCoreSim and
# in isolated HW probes (any payload size/address, all 7 XOR slots, incl. all
# cross-die lanes concurrently), but in the full kernel the cross-die (D2D)
# transfers lose their data payloads on repeated fills (sem bumps arrive, data
# doesn't), which desc-gen pacing did not cure.  Until that HW interaction is
# understood, the proven NRT-collective path ships.
USE_RDMA = False


def _get_nc(
    iteration: int, reps: int = 1, ablate: frozenset = frozenset()
) -> bass.Bass:
    use_rdma = USE_RDMA and _get_sigma() is not False
    key = (iteration, reps, ablate, use_rdma)
    if key not in _NC_CACHE:
        _NC_CACHE[key] = build(iteration, reps=reps, ablate=ablate, use_rdma=use_rdma)
    return _NC_CACHE[key]


def kernel(
    input,
    hidden,
    edge_matrix,
    ba,
    wz_wr_wh,
    uz_ur,
    uh,
    input_wzrh,
    bw,
    iteration,
):
    iteration = int(iteration)
    if iteration <= 0:
        return np.asarray(hidden, np.float32).copy()

    sigma = _get_sigma() if USE_RDMA else False
    nc = _get_nc(iteration)
    in_maps = _host_prep(
        input, hidden, edge_matrix, ba, wz_wr_wh, uz_ur, uh, input_wzrh, bw,
        sigma=None if sigma is False else sigma,
    )
    res = run_bass_kernel_spmd(nc, in_maps, list(range(M)))
    out = np.concatenate([res.results[m]["h_out"] for m in range(M)], axis=0)
    return out[None]
